# revision 4
# baseline (speedup 1.0000x reference)
"""Trainium2 Bass kernel for nn_AU_Net (GNN message passing).

Strategy (8 NeuronCores, SPMD):
- Nodes sharded 6250/core. Weights replicated.
- Host preprocessing (graph structure only): degree/norm factors, edge
  partitioning by destination core, destination-block bucketing, int16
  gather-index packing (table split in two halves to fit int16).
- Dense layers run feature-major (activations [128 feat, nodes]) so
  matmuls need no activation transposes.
- GCN aggregation: edges sorted by destination block (64 nodes); source
  rows fetched with dma_gather (bf16, 256B/row); scatter-add via one-hot
  matmul into PSUM (lhsT=messages stationary, rhs=onehot), producing
  feature-major block outputs.
- D^{-1/2}: dinv[src] folded into the gather tables (h' = dinv*h),
  dinv[dst] applied at PSUM evacuation.
- Conv1's table is computed replicated on every core (cheaper than an
  AllGather); conv2's table needs an AllGather of z1-derived rows.

kernel(**inputs) takes full unsharded inputs, returns the full output.
"""
import numpy as np
import ml_dtypes

import concourse.bass as bass
import concourse.bacc as bacc
import concourse.tile as tile
import concourse.mybir as mybir
from concourse import bass_utils

BF16 = mybir.dt.bfloat16
F32 = mybir.dt.float32
I16 = mybir.dt.int16

NCORES = 8
M = 64             # nodes per destination block
GROUP_BLOCKS = 4   # dst blocks per gather-call group
PAD_DSTL = 99.0    # sentinel within-block id for padding edges (>= M)


class Meta:
    pass


# ----------------------------------------------------------------------------
# Host preprocessing (graph structure only)
# ----------------------------------------------------------------------------

def preprocess(edge_index: np.ndarray, n_nodes: int):
    N = n_nodes
    NPC = N // NCORES
    assert NPC * NCORES == N
    TSPLIT = N // 2
    assert TSPLIT < 32768 and N - TSPLIT < 32768
    NBLK = (NPC + M - 1) // M

    src = np.asarray(edge_index[0], dtype=np.int64)
    dst = np.asarray(edge_index[1], dtype=np.int64)

    deg = np.bincount(dst, minlength=N).astype(np.float64) + 1.0
    dinv = (1.0 / np.sqrt(deg)).astype(np.float32)

    loops = np.arange(N, dtype=np.int64)
    src = np.concatenate([src, loops])
    dst = np.concatenate([dst, loops])

    core = dst // NPC
    dl = dst % NPC
    blk = dl // M
    within = (dl % M).astype(np.float32)
    stream = (src >= TSPLIT).astype(np.int64)

    key = (core * NBLK + blk) * 2 + stream
    order = np.argsort(key, kind="stable")
    src_s = src[order]
    within_s = within[order]
    key_s = key[order]

    counts = np.bincount(key_s, minlength=NCORES * NBLK * 2) \
        .reshape(NCORES, NBLK, 2)
    seg_start = np.concatenate([[0], np.cumsum(counts.reshape(-1))])[:-1] \
        .reshape(NCORES, NBLK, 2)

    # uniform col-block counts across cores
    cb = -(-counts.max(axis=0) // 128)          # [NBLK, 2]
    cb[:, 0] = np.maximum(cb[:, 0], 1)          # >=1 col-block per dst block

    ngroups = (NBLK + GROUP_BLOCKS - 1) // GROUP_BLOCKS
    groups = []
    cb_cursor = 0
    for g in range(ngroups):
        blocks = list(range(g * GROUP_BLOCKS, min((g + 1) * GROUP_BLOCKS, NBLK)))
        ginfo = {"blocks": blocks, "calls": []}
        for s in (0, 1):
            cbs = [int(cb[b, s]) for b in blocks]
            ginfo["calls"].append({
                "stream": s,
                "cb_total": sum(cbs),
                "cb_per_block": cbs,
                "cb_offset": cb_cursor,
            })
            cb_cursor += sum(cbs)
        groups.append(ginfo)
    CBTOT = cb_cursor

    meta = Meta()
    meta.N, meta.NPC, meta.NBLK, meta.TSPLIT = N, NPC, NBLK, TSPLIT
    meta.CBTOT = CBTOT
    meta.groups = groups

    idx16_all, dstl_all, dinv_own_all, dinv_bc_all = [], [], [], []
    for c in range(NCORES):
        idx_cols = np.zeros((16, CBTOT * 8), np.int16)
        dstl = np.full((128, CBTOT), PAD_DSTL, ml_dtypes.bfloat16)
        cbi = 0
        for g in groups:
            for call in g["calls"]:
                s = call["stream"]
                n_edges_call = call["cb_total"] * 128
                e_idx = np.zeros(n_edges_call, np.int64)
                e_dstl = np.full(n_edges_call, PAD_DSTL, np.float32)
                off = 0
                for b, ncb in zip(g["blocks"], call["cb_per_block"]):
                    s0 = seg_start[c, b, s]
                    cnt = counts[c, b, s]
                    e_idx[off:off + cnt] = src_s[s0:s0 + cnt]
                    e_dstl[off:off + cnt] = within_s[s0:s0 + cnt]
                    off += ncb * 128
                if s == 1:
                    e_idx = e_idx - TSPLIT
                    e_idx[e_idx < 0] = 0
                w = e_idx.astype(np.int16).reshape(-1, 16).T
                idx_cols[:, cbi * 8:cbi * 8 + call["cb_total"] * 8] = w
                dstl[:, cbi:cbi + call["cb_total"]] = \
                    e_dstl.reshape(-1, 128).T.astype(ml_dtypes.bfloat16)
                cbi += call["cb_total"]
        idx16_all.append(np.tile(idx_cols, (8, 1)))
        dstl_all.append(dstl)

        c0n = c * NPC
        nown = (NPC + 127) // 128
        dvals = dinv[c0n:c0n + NPC]
        d_own_flat = np.zeros(nown * 128, np.float32)
        d_own_flat[:NPC] = dvals
        dinv_own_all.append(np.ascontiguousarray(
            d_own_flat.reshape(nown, 128).T))
        dinv_bc_all.append(np.broadcast_to(
            dvals.astype(ml_dtypes.bfloat16)[None, :], (128, NPC)).copy())

    nnm = (N + 127) // 128
    d_nm_flat = np.zeros(nnm * 128, np.float32)
    d_nm_flat[:N] = dinv
    dinv_nm = np.ascontiguousarray(d_nm_flat.reshape(nnm, 128).T)

    arrays = {
        "idx16": idx16_all,
        "dstl": dstl_all,
        "dinv_own_nm": dinv_own_all,
        "dinv_bc": dinv_bc_all,
        "dinv_nm": dinv_nm,
    }
    return meta, arrays


# ----------------------------------------------------------------------------
# Device program
# ----------------------------------------------------------------------------

def build_program(meta):
    N, NPC, NBLK = meta.N, meta.NPC, meta.NBLK
    TSPLIT, CBTOT = meta.TSPLIT, meta.CBTOT
    NNM = (N + 127) // 128
    NOWN = (NPC + 127) // 128
    STRIP = 512
    groups = meta.groups
    max_cb = max(c["cb_total"] for g in groups for c in g["calls"])

    nc = bacc.Bacc("TRN2", target_bir_lowering=False, debug=False,
                   num_devices=NCORES, num_swdge_queues=4)

    def din(name, shape, dt):
        return nc.dram_tensor(name, shape, dt, kind="ExternalInput").ap()

    xT = din("xT", [128, N], BF16)
    gxT = din("gxT", [128, N], BF16)
    xTo = din("xTo", [128, NPC], BF16)
    gxTo = din("gxTo", [128, NPC], BF16)
    idx16 = din("idx16", [128, CBTOT * 8], I16)
    dstl_d = din("dstl", [128, CBTOT], BF16)
    dinv_nm_d = din("dinv_nm", [128, NNM], F32)
    dinv_own_d = din("dinv_own_nm", [128, NOWN], F32)
    dinv_bc_d = din("dinv_bc", [128, NPC], BF16)

    wspec = [("W1a", 128), ("W1b", 128), ("Wdr", 128), ("Wg1", 128),
             ("Wg2", 128), ("W2a", 128), ("W2b", 128), ("W2c", 128),
             ("W3", 64)]
    wins = {nm: din(nm, [128, fo], BF16) for nm, fo in wspec}
    Wo = din("Wo", [64, 10], BF16)
    bspec = [("b1", 128), ("bdr", 128), ("bg1", 128), ("bg2", 128),
             ("b2", 128), ("b3", 64), ("bo", 10)]
    bins = {nm: din(nm, [d, 1], F32) for nm, d in bspec}

    outT = nc.dram_tensor("outT", [10, NPC], F32, kind="ExternalOutput").ap()

    ADD, MAX, MULT = (mybir.AluOpType.add, mybir.AluOpType.max,
                      mybir.AluOpType.mult)

    with tile.TileContext(nc) as tc:
        with tc.tile_pool(name="res", bufs=1) as res, \
             tc.tile_pool(name="dram", bufs=1, space="DRAM") as dram, \
             tc.tile_pool(name="work", bufs=1) as work, \
             tc.tile_pool(name="pbig", bufs=3, space="PSUM") as pbig, \
             tc.tile_pool(name="pconv", bufs=3, space="PSUM") as pconv:

            # ---------------- residents ----------------
            zT = res.tile([128, NPC], BF16)
            z0T = res.tile([128, NPC], BF16)
            z1T = res.tile([128, NPC], BF16)
            z2T = res.tile([128, NPC], BF16)
            dstl_s = res.tile([128, CBTOT], BF16)
            dinv_nm_s = res.tile([128, NNM], F32)
            dinv_own_s = res.tile([128, NOWN], F32)
            dinv_bc_s = res.tile([128, NPC], BF16)
            iota64 = res.tile([128, M], BF16)

            nc.sync.dma_start(out=dstl_s[:], in_=dstl_d[:])
            nc.sync.dma_start(out=dinv_nm_s[:], in_=dinv_nm_d[:])
            nc.sync.dma_start(out=dinv_own_s[:], in_=dinv_own_d[:])
            nc.sync.dma_start(out=dinv_bc_s[:], in_=dinv_bc_d[:])
            nc.gpsimd.iota(iota64[:], pattern=[[1, M]], base=0,
                           channel_multiplier=0,
                           allow_small_or_imprecise_dtypes=True)

            wt = {}
            for nm, fo in wspec:
                t = res.tile([128, fo], BF16, name=f"w_{nm}")
                nc.sync.dma_start(out=t[:], in_=wins[nm][:])
                wt[nm] = t
            wo_t = res.tile([64, 10], BF16)
            nc.sync.dma_start(out=wo_t[:], in_=Wo[:])
            bias = {}
            for nm, d in bspec:
                t = res.tile([d, 1], F32, name=f"b_{nm}")
                nc.sync.dma_start(out=t[:], in_=bins[nm][:])
                bias[nm] = t

            # ---------------- DRAM internals ----------------
            table1 = dram.tile([N, 128], BF16)
            ag_in = dram.tile([NPC, 128], BF16)
            table2 = dram.tile([N, 128], BF16, addr_space="Shared")

            # ---------------- phase 1: replicated table1 build -------------
            nstrips = (N + STRIP - 1) // STRIP
            for si in range(nstrips):
                s0 = si * STRIP
                cols = min(STRIP, N - s0)
                xs = work.tile([128, STRIP], BF16, tag="xs", bufs=3)
                gs = work.tile([128, STRIP], BF16, tag="gs", bufs=3)
                nc.sync.dma_start(out=xs[:, :cols], in_=xT[:, s0:s0 + cols])
                nc.sync.dma_start(out=gs[:, :cols], in_=gxT[:, s0:s0 + cols])

                pz = pbig.tile([128, STRIP], F32, tag="big")
                nc.tensor.matmul(out=pz[:, :cols], lhsT=wt["W1a"][:],
                                 rhs=xs[:, :cols], start=True, stop=False)
                nc.tensor.matmul(out=pz[:, :cols], lhsT=wt["W1b"][:],
                                 rhs=gs[:, :cols], start=False, stop=True)
                zs = work.tile([128, STRIP], BF16, tag="zs", bufs=3)
                nc.vector.tensor_scalar(out=zs[:, :cols], in0=pz[:, :cols],
                                        scalar1=bias["b1"][:, :1], scalar2=0.0,
                                        op0=ADD, op1=MAX)
                a_s = work.tile([128, STRIP], BF16, tag="as", bufs=3)
                nc.vector.tensor_add(out=a_s[:, :cols], in0=zs[:, :cols],
                                     in1=gs[:, :cols])
                for k in range((cols + 127) // 128):
                    mcols = min(128, cols - k * 128)
                    pn = pconv.tile([128, 128], F32, tag="pnm", bufs=2)
                    nc.tensor.matmul(out=pn[:mcols, :],
                                     lhsT=a_s[:, k * 128:k * 128 + mcols],
                                     rhs=wt["Wg1"][:], start=True, stop=True)
                    hv = work.tile([128, 128], BF16, tag="hv", bufs=4)
                    gchunk = (s0 + k * 128) // 128
                    nc.vector.tensor_scalar(
                        out=hv[:mcols, :], in0=pn[:mcols, :],
                        scalar1=dinv_nm_s[:mcols, gchunk:gchunk + 1],
                        scalar2=None, op0=MULT)
                    nc.sync.dma_start(
                        out=table1[s0 + k * 128:s0 + k * 128 + mcols, :],
                        in_=hv[:mcols, :])

            # own-slice z and z0 (recomputed from own columns)
            nown_strips = (NPC + STRIP - 1) // STRIP
            for si in range(nown_strips):
                s0 = si * STRIP
                cols = min(STRIP, NPC - s0)
                xs2 = work.tile([128, STRIP], BF16, tag="xs2", bufs=2)
                gs2 = work.tile([128, STRIP], BF16, tag="gs2", bufs=2)
                nc.sync.dma_start(out=xs2[:, :cols], in_=xTo[:, s0:s0 + cols])
                nc.sync.dma_start(out=gs2[:, :cols], in_=gxTo[:, s0:s0 + cols])
                pz2 = pbig.tile([128, STRIP], F32, tag="big")
                nc.tensor.matmul(out=pz2[:, :cols], lhsT=wt["W1a"][:],
                                 rhs=xs2[:, :cols], start=True, stop=False)
                nc.tensor.matmul(out=pz2[:, :cols], lhsT=wt["W1b"][:],
                                 rhs=gs2[:, :cols], start=False, stop=True)
                nc.vector.tensor_scalar(out=zT[:, s0:s0 + cols],
                                        in0=pz2[:, :cols],
                                        scalar1=bias["b1"][:, :1], scalar2=0.0,
                                        op0=ADD, op1=MAX)
                pz0 = pbig.tile([128, STRIP], F32, tag="big")
                nc.tensor.matmul(out=pz0[:, :cols], lhsT=wt["Wdr"][:],
                                 rhs=zT[:, s0:s0 + cols], start=True, stop=True)
                nc.vector.tensor_scalar(out=z0T[:, s0:s0 + cols],
                                        in0=pz0[:, :cols],
                                        scalar1=bias["bdr"][:, :1],
                                        scalar2=None, op0=ADD)

            # ---------------- conv phases ----------------
            qn = [0]

            def conv_phase(table, out_res, bias_col):
                for g in groups:
                    msgs, ohs = [], []
                    for call in g["calls"]:
                        s = call["stream"]
                        ncb = call["cb_total"]
                        coff = call["cb_offset"]
                        nidx = ncb * 128
                        it = work.tile([128, max_cb * 8], I16,
                                       tag=f"idx{s}", bufs=2)
                        nc.sync.dma_start(
                            out=it[:, :ncb * 8],
                            in_=idx16[:, coff * 8:(coff + ncb) * 8])
                        mt = work.tile([128, max_cb, 128], BF16,
                                       tag=f"msg{s}", bufs=2)
                        src_ap = (table[0:TSPLIT, :] if s == 0
                                  else table[TSPLIT:N, :])
                        # dma_gather crashes above ~1024 idxs/call (SWDGE
                        # descriptor ring capacity) -- split into sub-calls,
                        # cycling the 4 SWDGE queues (parallel desc-gen)
                        SUBCB = 8
                        for k0 in range(0, ncb, SUBCB):
                            kcb = min(SUBCB, ncb - k0)
                            nc.gpsimd.dma_gather(
                                out_ap=mt[:, k0:k0 + kcb, :],
                                in_ap=src_ap,
                                idxs_ap=it[:, k0 * 8:(k0 + kcb) * 8],
                                num_idxs=kcb * 128,
                                num_idxs_reg=kcb * 128,
                                elem_size=128,
                                queue_num=qn[0] % 4,
                            )
                            qn[0] += 1
                        oh = work.tile([128, max_cb, M], BF16,
                                       tag=f"oh{s}", bufs=2)
                        iota_b = iota64[:].unsqueeze(1) \
                            .broadcast_to([128, ncb, M])
                        dstl_b = dstl_s[:, coff:coff + ncb].unsqueeze(2) \
                            .broadcast_to([128, ncb, M])
                        nc.vector.tensor_tensor(out=oh[:, :ncb, :],
                                                in0=iota_b, in1=dstl_b,
                                                op=mybir.AluOpType.is_equal)
                        msgs.append(mt)
                        ohs.append(oh)

                    colpos = [0, 0]
                    for bi, b in enumerate(g["blocks"]):
                        pc = pconv.tile([128, M], F32, tag="pcv", bufs=3)
                        ncb_a = g["calls"][0]["cb_per_block"][bi]
                        ncb_b = g["calls"][1]["cb_per_block"][bi]
                        tot = ncb_a + ncb_b
                        done = 0
                        for s, ncb_s in ((0, ncb_a), (1, ncb_b)):
                            for k in range(ncb_s):
                                col = colpos[s] + k
                                nc.tensor.matmul(
                                    out=pc[:],
                                    lhsT=msgs[s][:, col, :],
                                    rhs=ohs[s][:, col, :],
                                    start=(done == 0),
                                    stop=(done == tot - 1))
                                done += 1
                            colpos[s] += ncb_s
                        mb = min(M, NPC - b * M)
                        bcol = b * M
                        tmpv = work.tile([128, M], BF16, tag="cevac", bufs=4)
                        nc.vector.tensor_mul(
                            out=tmpv[:, :mb], in0=pc[:, :mb],
                            in1=dinv_bc_s[:, bcol:bcol + mb])
                        nc.vector.tensor_scalar(
                            out=out_res[:, bcol:bcol + mb],
                            in0=tmpv[:, :mb],
                            scalar1=bias_col[:, :1], scalar2=0.0,
                            op0=ADD, op1=MAX)

            conv_phase(table1, z1T, bias["bg1"])

            # ---------------- phase 3: h2' own + AllGather ------------------
            for k in range(NOWN):
                mcols = min(128, NPC - k * 128)
                pn2 = pconv.tile([128, 128], F32, tag="pnm", bufs=2)
                nc.tensor.matmul(out=pn2[:mcols, :],
                                 lhsT=z1T[:, k * 128:k * 128 + mcols],
                                 rhs=wt["Wg2"][:], start=True, stop=True)
                hv2 = work.tile([128, 128], BF16, tag="hv", bufs=4)
                nc.vector.tensor_scalar(
                    out=hv2[:mcols, :], in0=pn2[:mcols, :],
                    scalar1=dinv_own_s[:mcols, k:k + 1],
                    scalar2=None, op0=MULT)
                nc.sync.dma_start(out=ag_in[k * 128:k * 128 + mcols, :],
                                  in_=hv2[:mcols, :])
            nc.gpsimd.collective_compute(
                "AllGather",
                mybir.AluOpType.bypass,
                replica_groups=[list(range(NCORES))],
                ins=[ag_in[:]],
                outs=[table2[:]],
            )

            conv_phase(table2, z2T, bias["bg2"])

            # ---------------- phase 5: final dense layers (own) ------------
            for si in range(nown_strips):
                s0 = si * STRIP
                cols = min(STRIP, NPC - s0)
                pcc = pbig.tile([128, STRIP], F32, tag="big")
                nc.tensor.matmul(out=pcc[:, :cols], lhsT=wt["W2a"][:],
                                 rhs=zT[:, s0:s0 + cols],
                                 start=True, stop=False)
                nc.tensor.matmul(out=pcc[:, :cols], lhsT=wt["W2b"][:],
                                 rhs=z1T[:, s0:s0 + cols],
                                 start=False, stop=False)
                nc.tensor.matmul(out=pcc[:, :cols], lhsT=wt["W2c"][:],
                                 rhs=z2T[:, s0:s0 + cols],
                                 start=False, stop=True)
                zc = work.tile([128, STRIP], BF16, tag="zc", bufs=2)
                nc.vector.tensor_scalar(out=zc[:, :cols], in0=pcc[:, :cols],
                                        scalar1=bias["b2"][:, :1], scalar2=0.0,
                                        op0=ADD, op1=MAX)
                u = work.tile([128, STRIP], BF16, tag="u", bufs=2)
                nc.vector.tensor_add(out=u[:, :cols], in0=zc[:, :cols],
                                     in1=z0T[:, s0:s0 + cols])
                p6 = pbig.tile([64, STRIP], F32, tag="big")
                nc.tensor.matmul(out=p6[:, :cols], lhsT=wt["W3"][:],
                                 rhs=u[:, :cols], start=True, stop=True)
                u6 = work.tile([64, STRIP], BF16, tag="u6", bufs=2)
                nc.vector.tensor_scalar(out=u6[:, :cols], in0=p6[:, :cols],
                                        scalar1=bias["b3"][:, :1], scalar2=0.0,
                                        op0=ADD, op1=MAX)
                po = pbig.tile([10, STRIP], F32, tag="big")
                nc.tensor.matmul(out=po[:, :cols], lhsT=wo_t[:],
                                 rhs=u6[:, :cols], start=True, stop=True)
                ofin = work.tile([10, STRIP], F32, tag="ofin", bufs=2)
                nc.vector.tensor_scalar(out=ofin[:, :cols], in0=po[:, :cols],
                                        scalar1=bias["bo"][:, :1],
                                        scalar2=None, op0=ADD)
                nc.sync.dma_start(out=outT[:, s0:s0 + cols],
                                  in_=ofin[:, :cols])

    nc.compile()
    return nc


# ----------------------------------------------------------------------------
# Input packing + entry point
# ----------------------------------------------------------------------------

def pack_inputs(inputs, meta, arrays):
    NPC = meta.NPC
    bf = ml_dtypes.bfloat16

    x = np.asarray(inputs["x"], np.float32)
    gx = np.asarray(inputs["gx"], np.float32)
    xT = np.ascontiguousarray(x.T.astype(bf))
    gxT = np.ascontiguousarray(gx.T.astype(bf))

    W1 = np.asarray(inputs["W1"], np.float32)
    W2 = np.asarray(inputs["W2"], np.float32)
    weights = {
        "W1a": W1[:128].astype(bf), "W1b": W1[128:].astype(bf),
        "Wdr": np.asarray(inputs["Wdr"], np.float32).astype(bf),
        "Wg1": np.asarray(inputs["Wg1"], np.float32).astype(bf),
        "Wg2": np.asarray(inputs["Wg2"], np.float32).astype(bf),
        "W2a": W2[:128].astype(bf), "W2b": W2[128:256].astype(bf),
        "W2c": W2[256:].astype(bf),
        "W3": np.asarray(inputs["W3"], np.float32).astype(bf),
        "Wo": np.asarray(inputs["Wo"], np.float32).astype(bf),
    }
    biases = ["b1", "bdr", "bg1", "bg2", "b2", "b3", "bo"]

    in_maps = []
    for c in range(NCORES):
        m = {
            "xT": xT, "gxT": gxT,
            "xTo": np.ascontiguousarray(xT[:, c * NPC:(c + 1) * NPC]),
            "gxTo": np.ascontiguousarray(gxT[:, c * NPC:(c + 1) * NPC]),
            "idx16": arrays["idx16"][c],
            "dstl": arrays["dstl"][c],
            "dinv_nm": arrays["dinv_nm"],
            "dinv_own_nm": arrays["dinv_own_nm"][c],
            "dinv_bc": arrays["dinv_bc"][c],
        }
        for k, v in weights.items():
            m[k] = np.ascontiguousarray(v)
        for k in biases:
            m[k] = np.ascontiguousarray(
                np.asarray(inputs[k], np.float32).reshape(-1, 1))
        in_maps.append(m)
    return in_maps


_CACHE = {}


def prepare(inputs):
    edge_index = np.asarray(inputs["edge_index"])
    n = int(np.asarray(inputs["x"]).shape[0])
    key = (n, edge_index.shape[1], hash(edge_index.tobytes()))
    if key not in _CACHE:
        meta, arrays = preprocess(edge_index, n)
        nc = build_program(meta)
        _CACHE.clear()
        _CACHE[key] = (nc, meta, arrays)
    return _CACHE[key]


def kernel(**inputs) -> np.ndarray:
    nc, meta, arrays = prepare(inputs)
    in_maps = pack_inputs(inputs, meta, arrays)
    res = bass_utils.run_bass_kernel_spmd(
        nc, in_maps, core_ids=list(range(NCORES)))
    out = np.concatenate(
        [res.results[c]["outT"].T for c in range(NCORES)], axis=0)
    return out.astype(np.float32)


# revision 20
# speedup vs baseline: 132.7755x; 132.7755x over previous
"""Trainium2 Bass kernel for nn_AU_Net (GNN message passing).

Strategy (8 NeuronCores, SPMD):
- Nodes sharded 6250/core. Weights replicated.
- Host preprocessing (graph structure only): degree/norm factors, edge
  partitioning by destination core, destination-block bucketing, int16
  gather-index packing (table split in two halves to fit int16).
- Dense layers run feature-major (activations [128 feat, nodes]) so
  matmuls need no activation transposes.
- GCN aggregation: edges sorted by destination block (64 nodes); source
  rows fetched with dma_gather (bf16, 256B/row); scatter-add via one-hot
  matmul into PSUM (lhsT=messages stationary, rhs=onehot), producing
  feature-major block outputs.
- D^{-1/2}: dinv[src] folded into the gather tables (h' = dinv*h),
  dinv[dst] applied at PSUM evacuation.
- Conv1's table is computed replicated on every core (cheaper than an
  AllGather); conv2's table needs an AllGather of z1-derived rows.

kernel(**inputs) takes full unsharded inputs, returns the full output.
"""
import numpy as np
import ml_dtypes

import concourse.bass as bass
import concourse.bacc as bacc
import concourse.tile as tile
import concourse.mybir as mybir
from concourse import bass_utils

BF16 = mybir.dt.bfloat16
F32 = mybir.dt.float32
I16 = mybir.dt.int16

NCORES = 8
M = 64             # nodes per destination block
GROUP_BLOCKS = 4   # dst blocks per gather-call group
PAD_DSTL = 99.0    # sentinel within-block id for padding edges (>= M)


class Meta:
    pass


# ablation switches for performance attribution (timing-only builds)
ABLATE = {"copygather": False, "noag": False, "noonehot": False,
          "noconvmm": False, "skeleton": False}


# ----------------------------------------------------------------------------
# Host preprocessing (graph structure only)
# ----------------------------------------------------------------------------

def preprocess(edge_index: np.ndarray, n_nodes: int):
    N = n_nodes
    NPC = N // NCORES
    assert NPC * NCORES == N
    TSPLIT = N // 2
    assert TSPLIT < 32768 and N - TSPLIT < 32768
    NBLK = (NPC + M - 1) // M

    src = np.asarray(edge_index[0], dtype=np.int64)
    dst = np.asarray(edge_index[1], dtype=np.int64)

    deg = np.bincount(dst, minlength=N).astype(np.float64) + 1.0
    dinv = (1.0 / np.sqrt(deg)).astype(np.float32)

    loops = np.arange(N, dtype=np.int64)
    src = np.concatenate([src, loops])
    dst = np.concatenate([dst, loops])

    core = dst // NPC
    dl = dst % NPC
    blk = dl // M
    within = (dl % M).astype(np.float32)
    stream = (src >= TSPLIT).astype(np.int64)

    key = (core * NBLK + blk) * 2 + stream
    order = np.argsort(key, kind="stable")
    src_s = src[order]
    within_s = within[order]
    key_s = key[order]

    counts = np.bincount(key_s, minlength=NCORES * NBLK * 2) \
        .reshape(NCORES, NBLK, 2)
    seg_start = np.concatenate([[0], np.cumsum(counts.reshape(-1))])[:-1] \
        .reshape(NCORES, NBLK, 2)

    # uniform col-block counts across cores
    cb = -(-counts.max(axis=0) // 128)          # [NBLK, 2]
    cb[:, 0] = np.maximum(cb[:, 0], 1)          # >=1 col-block per dst block

    ngroups = (NBLK + GROUP_BLOCKS - 1) // GROUP_BLOCKS
    groups = []
    cb_cursor = 0
    for g in range(ngroups):
        blocks = list(range(g * GROUP_BLOCKS, min((g + 1) * GROUP_BLOCKS, NBLK)))
        ginfo = {"blocks": blocks, "calls": []}
        for s in (0, 1):
            cbs = [int(cb[b, s]) for b in blocks]
            ginfo["calls"].append({
                "stream": s,
                "cb_total": sum(cbs),
                "cb_per_block": cbs,
                "cb_offset": cb_cursor,
            })
            cb_cursor += sum(cbs)
        groups.append(ginfo)
    CBTOT = cb_cursor

    meta = Meta()
    meta.N, meta.NPC, meta.NBLK, meta.TSPLIT = N, NPC, NBLK, TSPLIT
    meta.CBTOT = CBTOT
    meta.groups = groups

    idx16_all, dstl_all, dinv_own_all, dinv_bc_all = [], [], [], []
    for c in range(NCORES):
        idx_cols = np.zeros((16, CBTOT * 8), np.int16)
        dstl = np.full((128, CBTOT), PAD_DSTL, ml_dtypes.bfloat16)
        cbi = 0
        for g in groups:
            for call in g["calls"]:
                s = call["stream"]
                n_edges_call = call["cb_total"] * 128
                e_idx = np.zeros(n_edges_call, np.int64)
                e_dstl = np.full(n_edges_call, PAD_DSTL, np.float32)
                off = 0
                for b, ncb in zip(g["blocks"], call["cb_per_block"]):
                    s0 = seg_start[c, b, s]
                    cnt = counts[c, b, s]
                    e_idx[off:off + cnt] = src_s[s0:s0 + cnt]
                    e_dstl[off:off + cnt] = within_s[s0:s0 + cnt]
                    off += ncb * 128
                if s == 1:
                    e_idx = e_idx - TSPLIT
                    e_idx[e_idx < 0] = 0
                w = e_idx.astype(np.int16).reshape(-1, 16).T
                idx_cols[:, cbi * 8:cbi * 8 + call["cb_total"] * 8] = w
                dstl[:, cbi:cbi + call["cb_total"]] = \
                    e_dstl.reshape(-1, 128).T.astype(ml_dtypes.bfloat16)
                cbi += call["cb_total"]
        idx16_all.append(np.tile(idx_cols, (8, 1)))
        dstl_all.append(dstl)

        c0n = c * NPC
        nown = (NPC + 127) // 128
        dvals = dinv[c0n:c0n + NPC]
        d_own_flat = np.zeros(nown * 128, np.float32)
        d_own_flat[:NPC] = dvals
        dinv_own_all.append(np.ascontiguousarray(
            d_own_flat.reshape(nown, 128).T))
        dinv_bc_all.append(np.broadcast_to(
            dvals.astype(ml_dtypes.bfloat16)[None, :], (128, NPC)).copy())

    nnm = (N + 127) // 128
    d_nm_flat = np.zeros(nnm * 128, np.float32)
    d_nm_flat[:N] = dinv
    dinv_nm = np.ascontiguousarray(d_nm_flat.reshape(nnm, 128).T)

    arrays = {
        "idx16": idx16_all,
        "dstl": dstl_all,
        "dinv_own_nm": dinv_own_all,
        "dinv_bc": dinv_bc_all,
        "dinv_nm": dinv_nm,
    }
    return meta, arrays


# ----------------------------------------------------------------------------
# Device program
# ----------------------------------------------------------------------------

def build_program(meta):
    N, NPC, NBLK = meta.N, meta.NPC, meta.NBLK
    TSPLIT, CBTOT = meta.TSPLIT, meta.CBTOT
    NNM = (N + 127) // 128
    NOWN = (NPC + 127) // 128
    STRIP = 512
    groups = meta.groups
    max_cb = max(c["cb_total"] for g in groups for c in g["calls"])

    nc = bacc.Bacc("TRN2", target_bir_lowering=False, debug=False,
                   num_devices=NCORES, num_swdge_queues=4,
                   dynamic_dma_scratch_size=65536)

    def din(name, shape, dt):
        return nc.dram_tensor(name, shape, dt, kind="ExternalInput").ap()

    xT = din("xT", [128, N], BF16)
    gxT = din("gxT", [128, N], BF16)
    xTo = din("xTo", [128, NPC], BF16)
    gxTo = din("gxTo", [128, NPC], BF16)
    idx16 = din("idx16", [128, CBTOT * 8], I16)
    dstl_d = din("dstl", [128, CBTOT], BF16)
    dinv_nm_d = din("dinv_nm", [128, NNM], F32)
    dinv_own_d = din("dinv_own_nm", [128, NOWN], F32)
    dinv_bc_d = din("dinv_bc", [128, NPC], BF16)

    wspec = [("W1a", 128), ("W1b", 128), ("Wdr", 128), ("Wg1", 128),
             ("Wg2", 128), ("W2a", 128), ("W2b", 128), ("W2c", 128),
             ("W3", 64)]
    wins = {nm: din(nm, [128, fo], BF16) for nm, fo in wspec}
    Wo = din("Wo", [64, 10], BF16)
    bspec = [("b1", 128), ("bdr", 128), ("bg1", 128), ("bg2", 128),
             ("b2", 128), ("b3", 64), ("bo", 10)]
    bins = {nm: din(nm, [d, 1], F32) for nm, d in bspec}

    outT = nc.dram_tensor("outT", [10, NPC], F32, kind="ExternalOutput").ap()

    ADD, MAX, MULT = (mybir.AluOpType.add, mybir.AluOpType.max,
                      mybir.AluOpType.mult)

    with tile.TileContext(nc) as tc:
        with tc.tile_pool(name="res", bufs=1) as res, \
             tc.tile_pool(name="dram", bufs=1, space="DRAM") as dram, \
             tc.tile_pool(name="work", bufs=1) as work, \
             tc.tile_pool(name="pbig", bufs=3, space="PSUM") as pbig, \
             tc.tile_pool(name="pconv", bufs=3, space="PSUM") as pconv:

            # ---------------- residents ----------------
            zT = res.tile([128, NPC], BF16)
            z0T = res.tile([128, NPC], BF16)
            z1T = res.tile([128, NPC], BF16)
            z2T = res.tile([128, NPC], BF16)
            dstl_s = res.tile([128, CBTOT], BF16)
            dinv_nm_s = res.tile([128, NNM], F32)
            dinv_own_s = res.tile([128, NOWN], F32)
            dinv_bc_s = res.tile([128, NPC], BF16)
            iota64 = res.tile([128, M], BF16)

            nc.sync.dma_start(out=dstl_s[:], in_=dstl_d[:])
            nc.sync.dma_start(out=dinv_nm_s[:], in_=dinv_nm_d[:])
            nc.sync.dma_start(out=dinv_own_s[:], in_=dinv_own_d[:])
            nc.sync.dma_start(out=dinv_bc_s[:], in_=dinv_bc_d[:])
            nc.gpsimd.iota(iota64[:], pattern=[[1, M]], base=0,
                           channel_multiplier=0,
                           allow_small_or_imprecise_dtypes=True)

            wt = {}
            for nm, fo in wspec:
                t = res.tile([128, fo], BF16, name=f"w_{nm}")
                nc.sync.dma_start(out=t[:], in_=wins[nm][:])
                wt[nm] = t
            wo_t = res.tile([64, 10], BF16)
            nc.sync.dma_start(out=wo_t[:], in_=Wo[:])
            bias = {}
            for nm, d in bspec:
                t = res.tile([d, 1], F32, name=f"b_{nm}")
                nc.sync.dma_start(out=t[:], in_=bins[nm][:])
                bias[nm] = t

            # ---------------- DRAM internals ----------------
            table1 = dram.tile([N, 128], BF16)
            ag_in = dram.tile([NPC, 128], BF16)
            table2 = dram.tile(
                [N, 128], BF16,
                addr_space="Local" if ABLATE["noag"] else "Shared")

            # ---------------- phase 1: replicated table1 build -------------
            nstrips = (N + STRIP - 1) // STRIP
            for si in range(nstrips):
                s0 = si * STRIP
                cols = min(STRIP, N - s0)
                xs = work.tile([128, STRIP], BF16, tag="xs", bufs=3)
                gs = work.tile([128, STRIP], BF16, tag="gs", bufs=3)
                eng0 = nc.sync if si % 2 == 0 else nc.scalar
                eng1 = nc.scalar if si % 2 == 0 else nc.sync
                eng0.dma_start(out=xs[:, :cols], in_=xT[:, s0:s0 + cols])
                eng1.dma_start(out=gs[:, :cols], in_=gxT[:, s0:s0 + cols])

                pz = pbig.tile([128, STRIP], F32, tag="big")
                nc.tensor.matmul(out=pz[:, :cols], lhsT=wt["W1a"][:],
                                 rhs=xs[:, :cols], start=True, stop=False)
                nc.tensor.matmul(out=pz[:, :cols], lhsT=wt["W1b"][:],
                                 rhs=gs[:, :cols], start=False, stop=True)
                zs = work.tile([128, STRIP], BF16, tag="zs", bufs=3)
                nc.vector.tensor_scalar(out=zs[:, :cols], in0=pz[:, :cols],
                                        scalar1=bias["b1"][:, :1], scalar2=0.0,
                                        op0=ADD, op1=MAX)
                a_s = work.tile([128, STRIP], BF16, tag="as", bufs=3)
                nc.vector.tensor_add(out=a_s[:, :cols], in0=zs[:, :cols],
                                     in1=gs[:, :cols])
                hv4 = work.tile([128, 4, 128], BF16, tag="hv", bufs=3)
                nchunk = (cols + 127) // 128
                for k in range(nchunk):
                    mcols = min(128, cols - k * 128)
                    pn = pconv.tile([128, 128], F32, tag="pnm", bufs=2)
                    nc.tensor.matmul(out=pn[:mcols, :],
                                     lhsT=a_s[:, k * 128:k * 128 + mcols],
                                     rhs=wt["Wg1"][:], start=True, stop=True)
                    gchunk = (s0 + k * 128) // 128
                    nc.vector.tensor_scalar(
                        out=hv4[:mcols, k, :], in0=pn[:mcols, :],
                        scalar1=dinv_nm_s[:mcols, gchunk:gchunk + 1],
                        scalar2=None, op0=MULT)
                c128 = (cols // 128) * 128
                eng = nc.sync if si % 2 == 0 else nc.scalar
                if c128:
                    eng.dma_start(
                        out=table1[s0:s0 + c128, :].rearrange(
                            "(c p) f -> p c f", p=128),
                        in_=hv4[:, :c128 // 128, :])
                if cols - c128:
                    eng.dma_start(
                        out=table1[s0 + c128:s0 + cols, :],
                        in_=hv4[:cols - c128, c128 // 128, :])

            # own-slice z and z0 (recomputed from own columns)
            nown_strips = (NPC + STRIP - 1) // STRIP
            for si in range(nown_strips):
                s0 = si * STRIP
                cols = min(STRIP, NPC - s0)
                xs2 = work.tile([128, STRIP], BF16, tag="xs2", bufs=2)
                gs2 = work.tile([128, STRIP], BF16, tag="gs2", bufs=2)
                nc.sync.dma_start(out=xs2[:, :cols], in_=xTo[:, s0:s0 + cols])
                nc.sync.dma_start(out=gs2[:, :cols], in_=gxTo[:, s0:s0 + cols])
                pz2 = pbig.tile([128, STRIP], F32, tag="big")
                nc.tensor.matmul(out=pz2[:, :cols], lhsT=wt["W1a"][:],
                                 rhs=xs2[:, :cols], start=True, stop=False)
                nc.tensor.matmul(out=pz2[:, :cols], lhsT=wt["W1b"][:],
                                 rhs=gs2[:, :cols], start=False, stop=True)
                nc.vector.tensor_scalar(out=zT[:, s0:s0 + cols],
                                        in0=pz2[:, :cols],
                                        scalar1=bias["b1"][:, :1], scalar2=0.0,
                                        op0=ADD, op1=MAX)
                pz0 = pbig.tile([128, STRIP], F32, tag="big")
                nc.tensor.matmul(out=pz0[:, :cols], lhsT=wt["Wdr"][:],
                                 rhs=zT[:, s0:s0 + cols], start=True, stop=True)
                nc.vector.tensor_scalar(out=z0T[:, s0:s0 + cols],
                                        in0=pz0[:, :cols],
                                        scalar1=bias["bdr"][:, :1],
                                        scalar2=None, op0=ADD)

            # ---------------- conv phases ----------------
            oh_static = None
            if ABLATE["noonehot"]:
                oh_static = res.tile([128, max_cb, M], BF16, name="oh_static")
                nc.vector.memset(oh_static[:], 0.0)
            if ABLATE["noconvmm"]:
                nc.vector.memset(z1T[:], 0.1)
                nc.vector.memset(z2T[:], 0.1)

            def conv_phase(table, out_res, bias_col):
                for g in groups:
                    msgs, ohs = [], []
                    for call in g["calls"]:
                        s = call["stream"]
                        ncb = call["cb_total"]
                        coff = call["cb_offset"]
                        nidx = ncb * 128
                        it = work.tile([128, max_cb * 8], I16,
                                       tag=f"idx{s}", bufs=2)
                        nc.sync.dma_start(
                            out=it[:, :ncb * 8],
                            in_=idx16[:, coff * 8:(coff + ncb) * 8])
                        mt = work.tile([128, max_cb, 128], BF16,
                                       tag=f"msg{s}", bufs=2)
                        src_ap = (table[0:TSPLIT, :] if s == 0
                                  else table[TSPLIT:N, :])
                        # dma_gather crashes above ~1024 idxs/call (SWDGE
                        # descriptor ring capacity) -- split into sub-calls,
                        # cycling the 4 SWDGE queues (parallel desc-gen)
                        SUBCB = 8
                        for k0 in range(0, ncb, SUBCB):
                            kcb = min(SUBCB, ncb - k0)
                            if ABLATE["copygather"]:
                                nc.sync.dma_start(
                                    out=mt[:, k0:k0 + kcb, :],
                                    in_=table[:kcb * 128, :].rearrange(
                                        "(c p) f -> p c f", p=128))
                                continue
                            nc.gpsimd.dma_gather(
                                out_ap=mt[:, k0:k0 + kcb, :],
                                in_ap=src_ap,
                                idxs_ap=it[:, k0 * 8:(k0 + kcb) * 8],
                                num_idxs=kcb * 128,
                                num_idxs_reg=kcb * 128,
                                elem_size=128,
                            )
                        if ABLATE["noonehot"]:
                            msgs.append(mt)
                            ohs.append(oh_static)
                            continue
                        oh = work.tile([128, max_cb, M], BF16,
                                       tag=f"oh{s}", bufs=2)
                        iota_b = iota64[:].unsqueeze(1) \
                            .broadcast_to([128, ncb, M])
                        dstl_b = dstl_s[:, coff:coff + ncb].unsqueeze(2) \
                            .broadcast_to([128, ncb, M])
                        nc.vector.tensor_tensor(out=oh[:, :ncb, :],
                                                in0=iota_b, in1=dstl_b,
                                                op=mybir.AluOpType.is_equal)
                        msgs.append(mt)
                        ohs.append(oh)

                    if ABLATE["noconvmm"]:
                        continue
                    colpos = [0, 0]
                    for bi, b in enumerate(g["blocks"]):
                        pc = pconv.tile([128, M], F32, tag="pcv", bufs=3)
                        ncb_a = g["calls"][0]["cb_per_block"][bi]
                        ncb_b = g["calls"][1]["cb_per_block"][bi]
                        tot = ncb_a + ncb_b
                        done = 0
                        for s, ncb_s in ((0, ncb_a), (1, ncb_b)):
                            for k in range(ncb_s):
                                col = colpos[s] + k
                                nc.tensor.matmul(
                                    out=pc[:],
                                    lhsT=msgs[s][:, col, :],
                                    rhs=ohs[s][:, col, :],
                                    start=(done == 0),
                                    stop=(done == tot - 1))
                                done += 1
                            colpos[s] += ncb_s
                        mb = min(M, NPC - b * M)
                        bcol = b * M
                        tmpv = work.tile([128, M], BF16, tag="cevac", bufs=4)
                        nc.vector.tensor_mul(
                            out=tmpv[:, :mb], in0=pc[:, :mb],
                            in1=dinv_bc_s[:, bcol:bcol + mb])
                        nc.vector.tensor_scalar(
                            out=out_res[:, bcol:bcol + mb],
                            in0=tmpv[:, :mb],
                            scalar1=bias_col[:, :1], scalar2=0.0,
                            op0=ADD, op1=MAX)

            conv_phase(table1, z1T, bias["bg1"])

            # ---------------- phase 3: h2' own + AllGather ------------------
            for k0 in range(0, NOWN, 4):
                kn = min(4, NOWN - k0)
                hv2 = work.tile([128, 4, 128], BF16, tag="hv", bufs=3)
                for kk in range(kn):
                    k = k0 + kk
                    mcols = min(128, NPC - k * 128)
                    pn2 = pconv.tile([128, 128], F32, tag="pnm", bufs=2)
                    nc.tensor.matmul(out=pn2[:mcols, :],
                                     lhsT=z1T[:, k * 128:k * 128 + mcols],
                                     rhs=wt["Wg2"][:], start=True, stop=True)
                    nc.vector.tensor_scalar(
                        out=hv2[:mcols, kk, :], in0=pn2[:mcols, :],
                        scalar1=dinv_own_s[:mcols, k:k + 1],
                        scalar2=None, op0=MULT)
                r0 = k0 * 128
                rows = min(NPC - r0, kn * 128)
                r128 = (rows // 128) * 128
                if r128:
                    nc.sync.dma_start(
                        out=ag_in[r0:r0 + r128, :].rearrange(
                            "(c p) f -> p c f", p=128),
                        in_=hv2[:, :r128 // 128, :])
                if rows - r128:
                    nc.sync.dma_start(
                        out=ag_in[r0 + r128:r0 + rows, :],
                        in_=hv2[:rows - r128, r128 // 128, :])
            if ABLATE["noag"]:
                tfill = work.tile([128, 4096 // 128, 128], BF16,
                                  tag="msg0", bufs=2)
                nc.vector.memset(tfill[:], 0.125)
                for k in range(0, N, 4096):
                    rows = min(4096, N - k)
                    r128 = (rows // 128) * 128
                    if r128:
                        nc.sync.dma_start(
                            out=table2[k:k + r128, :].rearrange(
                                "(c p) f -> p c f", p=128),
                            in_=tfill[:, :r128 // 128, :])
                    if rows - r128:
                        nc.sync.dma_start(
                            out=table2[k + r128:k + rows, :],
                            in_=tfill[:rows - r128, 0, :])
            else:
                nc.gpsimd.collective_compute(
                    "AllGather",
                    mybir.AluOpType.bypass,
                    replica_groups=[list(range(NCORES))],
                    ins=[ag_in[:]],
                    outs=[table2[:]],
                )

            conv_phase(table2, z2T, bias["bg2"])

            # ---------------- phase 5: final dense layers (own) ------------
            for si in range(nown_strips):
                s0 = si * STRIP
                cols = min(STRIP, NPC - s0)
                pcc = pbig.tile([128, STRIP], F32, tag="big")
                nc.tensor.matmul(out=pcc[:, :cols], lhsT=wt["W2a"][:],
                                 rhs=zT[:, s0:s0 + cols],
                                 start=True, stop=False)
                nc.tensor.matmul(out=pcc[:, :cols], lhsT=wt["W2b"][:],
                                 rhs=z1T[:, s0:s0 + cols],
                                 start=False, stop=False)
                nc.tensor.matmul(out=pcc[:, :cols], lhsT=wt["W2c"][:],
                                 rhs=z2T[:, s0:s0 + cols],
                                 start=False, stop=True)
                zc = work.tile([128, STRIP], BF16, tag="zc", bufs=2)
                nc.vector.tensor_scalar(out=zc[:, :cols], in0=pcc[:, :cols],
                                        scalar1=bias["b2"][:, :1], scalar2=0.0,
                                        op0=ADD, op1=MAX)
                u = work.tile([128, STRIP], BF16, tag="u", bufs=2)
                nc.vector.tensor_add(out=u[:, :cols], in0=zc[:, :cols],
                                     in1=z0T[:, s0:s0 + cols])
                p6 = pbig.tile([64, STRIP], F32, tag="big")
                nc.tensor.matmul(out=p6[:, :cols], lhsT=wt["W3"][:],
                                 rhs=u[:, :cols], start=True, stop=True)
                u6 = work.tile([64, STRIP], BF16, tag="u6", bufs=2)
                nc.vector.tensor_scalar(out=u6[:, :cols], in0=p6[:, :cols],
                                        scalar1=bias["b3"][:, :1], scalar2=0.0,
                                        op0=ADD, op1=MAX)
                po = pbig.tile([10, STRIP], F32, tag="big")
                nc.tensor.matmul(out=po[:, :cols], lhsT=wo_t[:],
                                 rhs=u6[:, :cols], start=True, stop=True)
                ofin = work.tile([10, STRIP], F32, tag="ofin", bufs=2)
                nc.vector.tensor_scalar(out=ofin[:, :cols], in0=po[:, :cols],
                                        scalar1=bias["bo"][:, :1],
                                        scalar2=None, op0=ADD)
                nc.sync.dma_start(out=outT[:, s0:s0 + cols],
                                  in_=ofin[:, :cols])

    _assign_gather_queues(nc)
    nc.compile()
    return nc


def _assign_gather_queues(nc):
    """Post-scheduling: route each gather to SWDGE queue (lane % 4), where
    lane is the DMASW semaphore lane Tile assigned. Lanes then never share
    a queue's increments, keeping per-lane FIFO semantics sound while the
    4 queues generate descriptors in parallel."""
    for bb in nc.main_func.blocks:
        for inst in bb.instructions:
            if isinstance(inst, mybir.InstDMAGatherAnt):
                si = inst.sync_info
                if not si or not si.on_update:
                    continue
                nm = si.on_update[0].ant_name or ""
                if nm.startswith("DMASW"):
                    lane = int(nm[5:].split("_")[0])
                    inst.queue_num = lane % 4


def build_skeleton(meta):
    """Same I/O signature as build_program, trivial body (floor measure)."""
    N, NPC = meta.N, meta.NPC
    CBTOT = meta.CBTOT
    NNM = (N + 127) // 128
    NOWN = (NPC + 127) // 128
    nc = bacc.Bacc("TRN2", target_bir_lowering=False, debug=False,
                   num_devices=NCORES, num_swdge_queues=4,
                   dynamic_dma_scratch_size=65536)

    def din(name, shape, dt):
        return nc.dram_tensor(name, shape, dt, kind="ExternalInput").ap()

    din("xT", [128, N], BF16)
    din("gxT", [128, N], BF16)
    din("xTo", [128, NPC], BF16)
    din("gxTo", [128, NPC], BF16)
    din("idx16", [128, CBTOT * 8], I16)
    din("dstl", [128, CBTOT], BF16)
    din("dinv_nm", [128, NNM], F32)
    din("dinv_own_nm", [128, NOWN], F32)
    din("dinv_bc", [128, NPC], BF16)
    for nm, fo in [("W1a", 128), ("W1b", 128), ("Wdr", 128), ("Wg1", 128),
                   ("Wg2", 128), ("W2a", 128), ("W2b", 128), ("W2c", 128),
                   ("W3", 64)]:
        din(nm, [128, fo], BF16)
    din("Wo", [64, 10], BF16)
    for nm, d in [("b1", 128), ("bdr", 128), ("bg1", 128), ("bg2", 128),
                  ("b2", 128), ("b3", 64), ("bo", 10)]:
        din(nm, [d, 1], F32)
    outT = nc.dram_tensor("outT", [10, NPC], F32, kind="ExternalOutput").ap()
    with tile.TileContext(nc) as tc:
        with tc.tile_pool(name="w", bufs=1) as w:
            t = w.tile([10, NPC], F32)
            nc.vector.memset(t[:], 0.0)
            nc.sync.dma_start(out=outT[:], in_=t[:])
    nc.compile()
    return nc


# ----------------------------------------------------------------------------
# Input packing + entry point
# ----------------------------------------------------------------------------

def pack_inputs(inputs, meta, arrays):
    NPC = meta.NPC
    bf = ml_dtypes.bfloat16

    x = np.asarray(inputs["x"], np.float32)
    gx = np.asarray(inputs["gx"], np.float32)
    xT = np.ascontiguousarray(x.T.astype(bf))
    gxT = np.ascontiguousarray(gx.T.astype(bf))

    W1 = np.asarray(inputs["W1"], np.float32)
    W2 = np.asarray(inputs["W2"], np.float32)
    weights = {
        "W1a": W1[:128].astype(bf), "W1b": W1[128:].astype(bf),
        "Wdr": np.asarray(inputs["Wdr"], np.float32).astype(bf),
        "Wg1": np.asarray(inputs["Wg1"], np.float32).astype(bf),
        "Wg2": np.asarray(inputs["Wg2"], np.float32).astype(bf),
        "W2a": W2[:128].astype(bf), "W2b": W2[128:256].astype(bf),
        "W2c": W2[256:].astype(bf),
        "W3": np.asarray(inputs["W3"], np.float32).astype(bf),
        "Wo": np.asarray(inputs["Wo"], np.float32).astype(bf),
    }
    biases = ["b1", "bdr", "bg1", "bg2", "b2", "b3", "bo"]

    in_maps = []
    for c in range(NCORES):
        m = {
            "xT": xT, "gxT": gxT,
            "xTo": np.ascontiguousarray(xT[:, c * NPC:(c + 1) * NPC]),
            "gxTo": np.ascontiguousarray(gxT[:, c * NPC:(c + 1) * NPC]),
            "idx16": arrays["idx16"][c],
            "dstl": arrays["dstl"][c],
            "dinv_nm": arrays["dinv_nm"],
            "dinv_own_nm": arrays["dinv_own_nm"][c],
            "dinv_bc": arrays["dinv_bc"][c],
        }
        for k, v in weights.items():
            m[k] = np.ascontiguousarray(v)
        for k in biases:
            m[k] = np.ascontiguousarray(
                np.asarray(inputs[k], np.float32).reshape(-1, 1))
        in_maps.append(m)
    return in_maps


_CACHE = {}


def prepare(inputs):
    edge_index = np.asarray(inputs["edge_index"])
    n = int(np.asarray(inputs["x"]).shape[0])
    key = (n, edge_index.shape[1], hash(edge_index.tobytes()))
    if key not in _CACHE:
        meta, arrays = preprocess(edge_index, n)
        nc = build_program(meta)
        _CACHE.clear()
        _CACHE[key] = (nc, meta, arrays)
    return _CACHE[key]


def kernel(**inputs) -> np.ndarray:
    nc, meta, arrays = prepare(inputs)
    in_maps = pack_inputs(inputs, meta, arrays)
    res = bass_utils.run_bass_kernel_spmd(
        nc, in_maps, core_ids=list(range(NCORES)))
    out = np.concatenate(
        [res.results[c]["outT"].T for c in range(NCORES)], axis=0)
    return out.astype(np.float32)
